# revision 15
# baseline (speedup 1.0000x reference)
"""CometAttention Trainium2 kernel (fp16 I/O).

Computes, for query [B, D] and values [B, S, D] (B=32, S=2048, D=1024, f32):
    w[b, s]   = (query[b] . values[b, s]) / sqrt(D)
    w         = softmax(w, axis=0)            # over the batch dim!
    out[b,s,:] = values[b,s,:] * w[b,s]

Sharding: S is split across 8 NeuronCores (softmax over B is local to each
(s) column, so an S-shard needs no collectives). Each core gets
values[:, c*256:(c+1)*256, :] plus the full query, and produces the matching
output shard; the host concatenates shards along S.

The problem is HBM-bandwidth bound (~360 GB/s/core), so the device I/O is
done in fp16: the host rounds query and VSCALE-prescaled values to fp16
(rel err <= 2^-11, see VSCALE below), the device reads/writes fp16 (halving
DMA traffic, 64 MB -> 32 MB per core), and the host upcasts the output to
f32 and divides VSCALE back out. The softmax chain stays in f32 on-chip;
measured end-to-end max rel err is 2.7e-3, well inside the 2e-2 gate.

Per-core layout: s-positions are processed in [128, jj, 1024] fp16 SBUF
tiles. Partition block si (32 partitions, one per batch) holds the jj
contiguous s-positions s0+jj*si .. s0+jj*si+jj-1 on the free dim, with d
innermost (16KB contiguous fp16 DMA runs); tile[si*32+b, j, :] =
values[b, s0+jj*si+j, :]. The batch-dim softmax denominator is one TensorE
matmul against a block-diagonal ones matrix, which both group-sums over b
and broadcasts the result back to all 32 partitions of each group.

Engine budget per [128, 1024] chunk (64 chunks/core), from the same
instruction cost model the Tile scheduler uses:
- DVE:      tensor_mul fp16 @2x (594ns) + fold-add 256 @2x (193ns)
            + tensor_scalar weight @4x (327ns)              -> ~72us total
- ScalarE:  Copy-with-accumulate over 768 fp16 (1012ns)      -> ~66us total
- DMA:      33.6 MB @ 360 B/ns                               -> ~93us total
so the kernel rides the DMA roofline. Loads go through the SP HWDGE ring,
stores through the ScalarE ring, so a store waiting on the softmax chain
never head-blocks later loads. One 128-partition DMA per unit (not 4 per-si
DMAs) keeps descriptor runs at 16KB and sequencer overhead low.
tensor_tensor_reduce is avoided (it faults on this hardware/runtime); the
dot-product reduction is DVE fold + ScalarE Copy-with-accumulate.
"""

import os

import numpy as np
from contextlib import ExitStack

# Defensive: recover NeuronCores left wedged by a previous crashed run.
os.environ.setdefault("NEURON_RT_RESET_CORES", "1")

B = 32
S = 2048
D = 1024
N_CORES = 8
S_SH = S // N_CORES        # 256 s-positions per core
SG = 128 // B              # 4 partition groups of 32
JJ = 8                     # chunks per DMA unit

# Host-side power-of-two prescale of `values` before the fp16 cast. Without
# it, |values| down to ~7.5e-8 land in the fp16 subnormal range where the
# ABSOLUTE quantization step (3e-8) dominates: output elements near the
# test's 1e-6 denominator floor then see ~3e-2 relative error. Scaling by
# 2^10 (exact) moves every input into the fp16 normal range (min scaled
# magnitude 7.6e-5 > 6.1e-5) and shrinks output quantization 1024x; the
# score math divides it back out via the Exp activation's scale input and
# the host divides the output by 2^10 after the upcast. Measured end-to-end
# max rel err: 2.7e-3 (gate: 2e-2). Scaled dot-product terms stay < 1.7e4,
# safely inside fp16 range (65504).
VSCALE = 1024.0

_CACHE: dict = {}


def _build_nc(jj: int = JJ, v_bufs: int = 5, prod_bufs: int = 4,
              fold: int = 256, wave: int | None = None, taper: tuple = (6, 2)):
    import concourse.bacc as bacc
    import concourse.mybir as mybir
    import concourse.tile as tile

    f16 = mybir.dt.float16
    f32 = mybir.dt.float32
    Act = mybir.ActivationFunctionType

    nc = bacc.Bacc(
        "TRN2",
        target_bir_lowering=False,
        debug=False,
        enable_asserts=False,
        num_devices=N_CORES,
    )
    values = nc.dram_tensor("values", [B, S_SH, D], f16, kind="ExternalInput")
    query = nc.dram_tensor("query", [B, D], f16, kind="ExternalInput")
    out = nc.dram_tensor("out", [B, S_SH, D], f16, kind="ExternalOutput")
    v_ap, q_ap, o_ap = values.ap(), query.ap(), out.ap()

    with tile.TileContext(nc) as tc, ExitStack() as ctx:
        singles = ctx.enter_context(tc.tile_pool(name="singles", bufs=1))
        vpool = ctx.enter_context(tc.tile_pool(name="vpool", bufs=v_bufs))
        prodpool = ctx.enter_context(tc.tile_pool(name="prodpool", bufs=prod_bufs))
        wpool = ctx.enter_context(tc.tile_pool(name="wpool", bufs=6))
        pspool = ctx.enter_context(tc.tile_pool(name="pspool", bufs=4, space="PSUM"))

        # qtile[si*32 + b, :] = query[b, :]. One broadcast DMA (0-stride si
        # dim), issued on the SP ring BEFORE the first values load: the DMA
        # engines are an exclusive resource, and if this transfer queues
        # behind multi-MB unit loads, every dot product stalls on it.
        qtile = singles.tile([128, D], f16)
        nc.sync.dma_start(
            out=qtile, in_=q_ap.unsqueeze(0).broadcast_to((SG, B, D))
        )

        # Block-diagonal ones matrix: A[k, m] = 1 iff k//32 == m//32.
        # matmul(out, A, e) then computes out[p, j] = sum_{b in group(p)} e[b, j],
        # i.e. the group sum broadcast back to every partition of the group.
        atile = singles.tile([128, 128], f32)
        nc.vector.memset(atile, 0.0)
        for g in range(SG):
            nc.vector.memset(atile[g * B : (g + 1) * B, g * B : (g + 1) * B], 1.0)

        inv_sqrt_d = 1.0 / (float(np.sqrt(D)) * VSCALE)

        def do_wave(vtile, ounit, ujj, j_lo, j_hi):
            """Weights + scale + store for chunk range [j_lo, j_hi) of a
            loaded vtile; ounit is the matching output AP (or None until the
            final wave of the unit, which stores the whole unit)."""
            nw = j_hi - j_lo
            # dot products: wraw[p, j] = sum_d v[p, j, d] * q[b(p), d]
            # DVE elementwise product (fp16, 2x mode) + a fold-add of the last
            # `fold` elements onto the first `fold` (2x), then ScalarE
            # Copy-with-accumulate over the remaining D-fold elements.
            # (tensor_tensor_reduce faults on this HW; this split also
            # balances DVE vs ScalarE occupancy under the DMA roofline.)
            wraw = wpool.tile([128, nw], f32, tag="wraw")
            for j in range(j_lo, j_hi):
                prod = prodpool.tile([128, D], f16, tag="prod")
                nc.vector.tensor_mul(prod, vtile[:, j, :], qtile)
                if fold:
                    nc.vector.tensor_add(
                        prod[:, 0:fold], prod[:, 0:fold], prod[:, D - fold : D]
                    )
                red = prod[:, 0 : D - fold] if fold else prod
                nc.scalar.activation(
                    red,
                    red,
                    Act.Copy,
                    accum_out=wraw[:, j - j_lo : j - j_lo + 1],
                )

            # softmax over b (within each group of 32 partitions); the
            # 1/sqrt(D) score scale rides the Exp activation's scale input.
            etile = wpool.tile([128, nw], f32, tag="etile")
            nc.scalar.activation(etile, wraw, Act.Exp, scale=inv_sqrt_d)
            den = pspool.tile([128, nw], f32, tag="den")
            nc.tensor.matmul(den, atile, etile, start=True, stop=True)
            rec = wpool.tile([128, nw], f32, tag="rec")
            nc.vector.reciprocal(rec, den)
            wfin = wpool.tile([128, nw], f32, tag="wfin")
            nc.vector.tensor_mul(wfin, etile, rec)

            # scale values by the per-(b, s) weight, in place (vtile's last
            # reader is the dot-product mul, which already ran), and store.
            # fp16 tensor_scalar runs in the DVE 4x perf mode (the [128,1]
            # f32 scalar operand is exempt from the 2-byte requirement).
            for j in range(j_lo, j_hi):
                nc.vector.tensor_scalar_mul(
                    vtile[:, j, :], vtile[:, j, :],
                    wfin[:, j - j_lo : j - j_lo + 1]
                )
            if ounit is not None:
                # One 128-partition store for the whole unit on the ScalarE
                # HWDGE ring (qActDynamicHW) so its semaphore waits cannot
                # head-block the loads flowing through the SP ring.
                nc.scalar.dma_start(out=ounit, in_=vtile[:, 0:ujj, :])

        # unit sizes: uniform jj chunks, except tapered tail units (smaller
        # final units shorten the post-last-load compute+store tail)
        s_total = S_SH // SG  # total chunks per core
        tail = sum(taper)
        assert (s_total - tail) % jj == 0
        sizes = [jj] * ((s_total - tail) // jj) + [t for t in taper if t]

        s0 = 0
        for ujj in sizes:
            vtile = vpool.tile([128, jj, D], f16, tag="vtile")
            # One DMA for the whole unit: AP dims (si, b, j, d) with (j d)
            # contiguous -> 128 descriptors of jj*D*2 = 16KB each.
            vunit = v_ap[:, s0 : s0 + SG * ujj, :].rearrange(
                "b (si j) d -> si b j d", si=SG, j=ujj
            )
            ounit = o_ap[:, s0 : s0 + SG * ujj, :].rearrange(
                "b (si j) d -> si b j d", si=SG, j=ujj
            )
            nc.sync.dma_start(out=vtile[:, 0:ujj, :], in_=vunit)
            uw = wave or ujj
            for j_lo in range(0, ujj, uw):
                j_hi = min(j_lo + uw, ujj)
                do_wave(vtile, ounit if j_hi == ujj else None, ujj, j_lo, j_hi)
            s0 += SG * ujj

    nc.compile()
    return nc


def _get_nc():
    if "nc" not in _CACHE:
        _CACHE["nc"] = _build_nc()
    return _CACHE["nc"]


def kernel(query: np.ndarray, values: np.ndarray) -> np.ndarray:
    from concourse import bass_utils

    nc = _get_nc()
    # fp16 I/O: round-to-nearest on the host (values ~ N(0,1), far inside
    # fp16 range), upcast the result back to f32 after the gather.
    query = np.ascontiguousarray(np.asarray(query).astype(np.float16))
    values16 = (np.asarray(values) * np.float32(VSCALE)).astype(np.float16)
    in_maps = [
        {
            "values": np.ascontiguousarray(
                values16[:, c * S_SH : (c + 1) * S_SH, :]
            ),
            "query": query,
        }
        for c in range(N_CORES)
    ]
    last_exc = None
    for attempt in range(3):
        try:
            res = bass_utils.run_bass_kernel_spmd(
                nc, in_maps, core_ids=list(range(N_CORES))
            )
            out16 = np.concatenate([r["out"] for r in res.results], axis=1)
            return out16.astype(np.float32) * np.float32(1.0 / VSCALE)
        except ModuleNotFoundError:
            # BASS_TRACE=1 requests NTFF profiling, whose axon hook module is
            # not shipped in every container; fall back to an untraced run.
            os.environ["BASS_NEVER_TRACE"] = "1"
            last_exc = None
            continue
        except Exception as e:
            # A crashed previous run can leave a NeuronCore transiently
            # wedged (NRT_EXEC_UNIT_UNRECOVERABLE); NEURON_RT_RESET_CORES=1
            # recovers it on a fresh NRT session. Best effort: drop the jax
            # backend so the retry reconnects, and give the previous
            # session's teardown time to finish.
            last_exc = e
            import time as _time

            try:
                import jax.extend as _jex

                _jex.backend.clear_backends()
            except Exception:
                pass
            _time.sleep(20.0)
    raise last_exc


# revision 24
# speedup vs baseline: 1.0017x; 1.0017x over previous
"""CometAttention Trainium2 kernel (fp16 I/O).

Computes, for query [B, D] and values [B, S, D] (B=32, S=2048, D=1024, f32):
    w[b, s]   = (query[b] . values[b, s]) / sqrt(D)
    w         = softmax(w, axis=0)            # over the batch dim!
    out[b,s,:] = values[b,s,:] * w[b,s]

Sharding: S is split across 8 NeuronCores (softmax over B is local to each
(s) column, so an S-shard needs no collectives). Each core gets
values[:, c*256:(c+1)*256, :] plus the full query, and produces the matching
output shard; the host concatenates shards along S.

The problem is HBM-bandwidth bound (~360 GB/s/core), so the device I/O is
done in fp16: the host rounds query and VSCALE-prescaled values to fp16
(rel err <= 2^-11, see VSCALE below), the device reads/writes fp16 (halving
DMA traffic, 64 MB -> 32 MB per core), and the host upcasts the output to
f32 and divides VSCALE back out. The softmax chain stays in f32 on-chip;
measured end-to-end max rel err is 2.7e-3, well inside the 2e-2 gate.

Per-core layout: s-positions are processed in [128, jj, 1024] fp16 SBUF
tiles. Partition block si (32 partitions, one per batch) holds the jj
contiguous s-positions s0+jj*si .. s0+jj*si+jj-1 on the free dim, with d
innermost (16KB contiguous fp16 DMA runs); tile[si*32+b, j, :] =
values[b, s0+jj*si+j, :]. The batch-dim softmax denominator is one TensorE
matmul against a block-diagonal ones matrix, which both group-sums over b
and broadcasts the result back to all 32 partitions of each group.

Engine budget per [128, 1024] chunk (64 chunks/core), from the same
instruction cost model the Tile scheduler uses:
- DVE:      tensor_mul fp16 @2x (594ns) + fold-add 256 @2x (193ns)
            + tensor_scalar weight @4x (327ns)              -> ~72us total
- ScalarE:  Copy-with-accumulate over 768 fp16 (1012ns)      -> ~66us total
- DMA:      33.6 MB @ 360 B/ns                               -> ~93us total
so the kernel rides the DMA roofline. Loads go through the SP HWDGE ring,
stores through the ScalarE ring, so a store waiting on the softmax chain
never head-blocks later loads. One 128-partition DMA per unit (not 4 per-si
DMAs) keeps descriptor runs at 16KB and sequencer overhead low.
tensor_tensor_reduce is avoided (it faults on this hardware/runtime); the
dot-product reduction is DVE fold + ScalarE Copy-with-accumulate.
"""

import os

import numpy as np
from contextlib import ExitStack

# Defensive: recover NeuronCores left wedged by a previous crashed run.
os.environ.setdefault("NEURON_RT_RESET_CORES", "1")

B = 32
S = 2048
D = 1024
N_CORES = 8
S_SH = S // N_CORES        # 256 s-positions per core
SG = 128 // B              # 4 partition groups of 32
JJ = 8                     # chunks per DMA unit

# Host-side power-of-two prescale of `values` before the fp16 cast. Without
# it, |values| down to ~7.5e-8 land in the fp16 subnormal range where the
# ABSOLUTE quantization step (3e-8) dominates: output elements near the
# test's 1e-6 denominator floor then see ~3e-2 relative error. Scaling by
# 2^10 (exact) moves every input into the fp16 normal range (min scaled
# magnitude 7.6e-5 > 6.1e-5) and shrinks output quantization 1024x; the
# score math divides it back out via the Exp activation's scale input and
# the host divides the output by 2^10 after the upcast. Measured end-to-end
# max rel err: 2.7e-3 (gate: 2e-2). Scaled dot-product terms stay < 1.7e4,
# safely inside fp16 range (65504).
VSCALE = 1024.0

_CACHE: dict = {}


def _build_nc(jj: int = JJ, v_bufs: int = 5, prod_bufs: int = 4,
              fold: int = 384, wave: int | None = None, taper: tuple = (6, 1, 1),
              fold_engine: str = "pool", front: tuple = (1, 7)):
    import concourse.bacc as bacc
    import concourse.mybir as mybir
    import concourse.tile as tile

    f16 = mybir.dt.float16
    f32 = mybir.dt.float32
    Act = mybir.ActivationFunctionType

    nc = bacc.Bacc(
        "TRN2",
        target_bir_lowering=False,
        debug=False,
        enable_asserts=False,
        num_devices=N_CORES,
    )
    values = nc.dram_tensor("values", [B, S_SH, D], f16, kind="ExternalInput")
    query = nc.dram_tensor("query", [B, D], f16, kind="ExternalInput")
    out = nc.dram_tensor("out", [B, S_SH, D], f16, kind="ExternalOutput")
    v_ap, q_ap, o_ap = values.ap(), query.ap(), out.ap()

    with tile.TileContext(nc) as tc, ExitStack() as ctx:
        singles = ctx.enter_context(tc.tile_pool(name="singles", bufs=1))
        vpool = ctx.enter_context(tc.tile_pool(name="vpool", bufs=v_bufs))
        prodpool = ctx.enter_context(tc.tile_pool(name="prodpool", bufs=prod_bufs))
        wpool = ctx.enter_context(tc.tile_pool(name="wpool", bufs=6))
        pspool = ctx.enter_context(tc.tile_pool(name="pspool", bufs=4, space="PSUM"))
        qpool = ctx.enter_context(tc.tile_pool(name="qpool", bufs=2, space="PSUM"))

        # qtile[si*32 + b, :] = query[b, :]. The query is loaded from HBM
        # ONCE ([32, D], 0.18us of DMA, issued before the first values load
        # so it heads the DMA queue) and replicated 32->128 partitions
        # on-chip with one PE matmul against R = [I32 I32 I32 I32] (built by
        # GPSIMD affine_select: keep 1.0 where partition k == m%32). Reading
        # the 4x replica straight from HBM would cost 0.73us on the
        # exclusive DMA engines, which are the end-to-end bottleneck. The
        # broadcast finishes by ~4.5us, well before the first values unit
        # lands (~9.3us), so compute timing is unchanged.
        qtile = singles.tile([128, D], f16)
        q32 = singles.tile([B, D], f16)
        rtile = singles.tile([B, SG, B], f16)
        nc.gpsimd.memset(rtile, 1.0)
        nc.gpsimd.affine_select(
            rtile,
            rtile,
            pattern=[[0, SG], [-1, B]],
            compare_op=mybir.AluOpType.is_equal,
            fill=0.0,
            base=0,
            channel_multiplier=1,
        )
        nc.sync.dma_start(out=q32, in_=q_ap)
        half = D // 2
        for h in range(2):
            qb = qpool.tile([128, half], f32, tag="qb")
            nc.tensor.matmul(
                qb, rtile, q32[:, h * half : (h + 1) * half],
                start=True, stop=True,
            )
            nc.vector.tensor_copy(qtile[:, h * half : (h + 1) * half], qb)

        # Block-diagonal ones matrix: A[k, m] = 1 iff k//32 == m//32.
        # matmul(out, A, e) then computes out[p, j] = sum_{b in group(p)} e[b, j],
        # i.e. the group sum broadcast back to every partition of the group.
        atile = singles.tile([128, 128], f32)
        nc.vector.memset(atile, 0.0)
        for g in range(SG):
            nc.vector.memset(atile[g * B : (g + 1) * B, g * B : (g + 1) * B], 1.0)

        inv_sqrt_d = 1.0 / (float(np.sqrt(D)) * VSCALE)

        def do_wave(vtile, ounit, ujj, j_lo, j_hi):
            """Weights + scale + store for chunk range [j_lo, j_hi) of a
            loaded vtile; ounit is the matching output AP (or None until the
            final wave of the unit, which stores the whole unit)."""
            nw = j_hi - j_lo
            # dot products: wraw[p, j] = sum_d v[p, j, d] * q[b(p), d]
            # DVE elementwise product (fp16, 2x mode) + a fold-add of the last
            # `fold` elements onto the first `fold` (2x), then ScalarE
            # Copy-with-accumulate over the remaining D-fold elements.
            # (tensor_tensor_reduce faults on this HW; this split also
            # balances DVE vs ScalarE occupancy under the DMA roofline.)
            wraw = wpool.tile([128, nw], f32, tag="wraw")
            for j in range(j_lo, j_hi):
                prod = prodpool.tile([128, D], f16, tag="prod")
                nc.vector.tensor_mul(prod, vtile[:, j, :], qtile)
                if fold:
                    # the fold runs on the otherwise-idle GPSIMD engine (Add
                    # at 0.42 efficiency is still cheap), keeping both DVE
                    # and ScalarE below the per-unit DMA cadence
                    eng = nc.gpsimd if fold_engine == "pool" else nc.vector
                    eng.tensor_add(
                        prod[:, 0:fold], prod[:, 0:fold], prod[:, D - fold : D]
                    )
                red = prod[:, 0 : D - fold] if fold else prod
                nc.scalar.activation(
                    red,
                    red,
                    Act.Copy,
                    accum_out=wraw[:, j - j_lo : j - j_lo + 1],
                )

            # softmax over b (within each group of 32 partitions); the
            # 1/sqrt(D) score scale rides the Exp activation's scale input.
            etile = wpool.tile([128, nw], f32, tag="etile")
            nc.scalar.activation(etile, wraw, Act.Exp, scale=inv_sqrt_d)
            den = pspool.tile([128, nw], f32, tag="den")
            nc.tensor.matmul(den, atile, etile, start=True, stop=True)
            rec = wpool.tile([128, nw], f32, tag="rec")
            nc.vector.reciprocal(rec, den)
            wfin = wpool.tile([128, nw], f32, tag="wfin")
            nc.vector.tensor_mul(wfin, etile, rec)

            # scale values by the per-(b, s) weight, in place (vtile's last
            # reader is the dot-product mul, which already ran), and store.
            # fp16 tensor_scalar runs in the DVE 4x perf mode (the [128,1]
            # f32 scalar operand is exempt from the 2-byte requirement).
            for j in range(j_lo, j_hi):
                nc.vector.tensor_scalar_mul(
                    vtile[:, j, :], vtile[:, j, :],
                    wfin[:, j - j_lo : j - j_lo + 1]
                )
            if ounit is not None:
                # One 128-partition store for the whole unit on the ScalarE
                # HWDGE ring (qActDynamicHW) so its semaphore waits cannot
                # head-block the loads flowing through the SP ring.
                nc.scalar.dma_start(out=ounit, in_=vtile[:, 0:ujj, :])

        # unit sizes: uniform jj chunks, except tapered tail units (smaller
        # final units shorten the post-last-load compute+store tail)
        s_total = S_SH // SG  # total chunks per core
        tail = sum(taper) + sum(front)
        assert (s_total - tail) % jj == 0
        sizes = (list(front) + [jj] * ((s_total - tail) // jj)
                 + [t for t in taper if t])

        s0 = 0
        for ujj in sizes:
            vtile = vpool.tile([128, jj, D], f16, tag="vtile")
            # One DMA for the whole unit: AP dims (si, b, j, d) with (j d)
            # contiguous -> 128 descriptors of jj*D*2 = 16KB each.
            vunit = v_ap[:, s0 : s0 + SG * ujj, :].rearrange(
                "b (si j) d -> si b j d", si=SG, j=ujj
            )
            ounit = o_ap[:, s0 : s0 + SG * ujj, :].rearrange(
                "b (si j) d -> si b j d", si=SG, j=ujj
            )
            nc.sync.dma_start(out=vtile[:, 0:ujj, :], in_=vunit)
            uw = wave or ujj
            for j_lo in range(0, ujj, uw):
                j_hi = min(j_lo + uw, ujj)
                do_wave(vtile, ounit if j_hi == ujj else None, ujj, j_lo, j_hi)
            s0 += SG * ujj

    nc.compile()
    return nc


def _get_nc():
    if "nc" not in _CACHE:
        _CACHE["nc"] = _build_nc()
    return _CACHE["nc"]


def kernel(query: np.ndarray, values: np.ndarray) -> np.ndarray:
    from concourse import bass_utils

    nc = _get_nc()
    # fp16 I/O: round-to-nearest on the host (values ~ N(0,1), far inside
    # fp16 range), upcast the result back to f32 after the gather.
    query = np.ascontiguousarray(np.asarray(query).astype(np.float16))
    values16 = (np.asarray(values) * np.float32(VSCALE)).astype(np.float16)
    in_maps = [
        {
            "values": np.ascontiguousarray(
                values16[:, c * S_SH : (c + 1) * S_SH, :]
            ),
            "query": query,
        }
        for c in range(N_CORES)
    ]
    last_exc = None
    for attempt in range(3):
        try:
            res = bass_utils.run_bass_kernel_spmd(
                nc, in_maps, core_ids=list(range(N_CORES))
            )
            out16 = np.concatenate([r["out"] for r in res.results], axis=1)
            return out16.astype(np.float32) * np.float32(1.0 / VSCALE)
        except ModuleNotFoundError:
            # BASS_TRACE=1 requests NTFF profiling, whose axon hook module is
            # not shipped in every container; fall back to an untraced run.
            os.environ["BASS_NEVER_TRACE"] = "1"
            last_exc = None
            continue
        except Exception as e:
            # A crashed previous run can leave a NeuronCore transiently
            # wedged (NRT_EXEC_UNIT_UNRECOVERABLE); NEURON_RT_RESET_CORES=1
            # recovers it on a fresh NRT session. Best effort: drop the jax
            # backend so the retry reconnects, and give the previous
            # session's teardown time to finish.
            last_exc = e
            import time as _time

            try:
                import jax.extend as _jex

                _jex.backend.clear_backends()
            except Exception:
                pass
            _time.sleep(20.0)
    raise last_exc


# revision 31
# speedup vs baseline: 1.0066x; 1.0048x over previous
"""CometAttention Trainium2 kernel (fp16 I/O).

Computes, for query [B, D] and values [B, S, D] (B=32, S=2048, D=1024, f32):
    w[b, s]   = (query[b] . values[b, s]) / sqrt(D)
    w         = softmax(w, axis=0)            # over the batch dim!
    out[b,s,:] = values[b,s,:] * w[b,s]

Sharding: S is split across 8 NeuronCores (softmax over B is local to each
(s) column, so an S-shard needs no collectives). Each core gets
values[:, c*256:(c+1)*256, :] plus the full query, and produces the matching
output shard; the host concatenates shards along S.

The problem is HBM-bandwidth bound (~360 GB/s/core), so the device I/O is
done in fp16: the host rounds query and VSCALE-prescaled values to fp16
(rel err <= 2^-11, see VSCALE below), the device reads/writes fp16 (halving
DMA traffic, 64 MB -> 32 MB per core), and the host upcasts the output to
f32 and divides VSCALE back out. The softmax chain stays in f32 on-chip;
measured end-to-end max rel err is 2.7e-3, well inside the 2e-2 gate.

Per-core layout: s-positions are processed in [128, jj, 1024] fp16 SBUF
tiles. Partition block si (32 partitions, one per batch) holds the jj
contiguous s-positions s0+jj*si .. s0+jj*si+jj-1 on the free dim, with d
innermost (16KB contiguous fp16 DMA runs); tile[si*32+b, j, :] =
values[b, s0+jj*si+j, :]. The batch-dim softmax denominator is one TensorE
matmul against a block-diagonal ones matrix, which both group-sums over b
and broadcasts the result back to all 32 partitions of each group.

Engine budget per [128, 1024] chunk (64 chunks/core), from the same
instruction cost model the Tile scheduler uses:
- DVE:      tensor_mul fp16 @2x (594ns) + fold-add 256 @2x (193ns)
            + tensor_scalar weight @4x (327ns)              -> ~72us total
- ScalarE:  Copy-with-accumulate over 768 fp16 (1012ns)      -> ~66us total
- DMA:      33.6 MB @ 360 B/ns                               -> ~93us total
so the kernel rides the DMA roofline. Loads go through the SP HWDGE ring,
stores through the ScalarE ring, so a store waiting on the softmax chain
never head-blocks later loads. One 128-partition DMA per unit (not 4 per-si
DMAs) keeps descriptor runs at 16KB and sequencer overhead low.
tensor_tensor_reduce is avoided (it faults on this hardware/runtime); the
dot-product reduction is DVE fold + ScalarE Copy-with-accumulate.
"""

import os

import numpy as np
from contextlib import ExitStack

# Defensive: recover NeuronCores left wedged by a previous crashed run.
os.environ.setdefault("NEURON_RT_RESET_CORES", "1")

B = 32
S = 2048
D = 1024
N_CORES = 8
S_SH = S // N_CORES        # 256 s-positions per core
SG = 128 // B              # 4 partition groups of 32
JJ = 8                     # chunks per DMA unit

# Host-side power-of-two prescale of `values` before the fp16 cast. Without
# it, |values| down to ~7.5e-8 land in the fp16 subnormal range where the
# ABSOLUTE quantization step (3e-8) dominates: output elements near the
# test's 1e-6 denominator floor then see ~3e-2 relative error. Scaling by
# 2^10 (exact) moves every input into the fp16 normal range (min scaled
# magnitude 7.6e-5 > 6.1e-5) and shrinks output quantization 1024x; the
# score math divides it back out via the Exp activation's scale input and
# the host divides the output by 2^10 after the upcast. Measured end-to-end
# max rel err: 2.7e-3 (gate: 2e-2). Scaled dot-product terms stay < 1.7e4,
# safely inside fp16 range (65504).
VSCALE = 1024.0

_CACHE: dict = {}


def _build_nc(jj: int = JJ, v_bufs: int = 6, prod_bufs: int = 4,
              fold: int = 384, wave: int | None = None, taper: tuple = (6, 1, 1),
              fold_engine: str = "pool", front: tuple = (1, 7), qpos: int = 2):
    import concourse.bacc as bacc
    import concourse.mybir as mybir
    import concourse.tile as tile

    f16 = mybir.dt.float16
    f32 = mybir.dt.float32
    Act = mybir.ActivationFunctionType

    nc = bacc.Bacc(
        "TRN2",
        target_bir_lowering=False,
        debug=False,
        enable_asserts=False,
        num_devices=N_CORES,
    )
    values = nc.dram_tensor("values", [B, S_SH, D], f16, kind="ExternalInput")
    query = nc.dram_tensor("query", [B, D], f16, kind="ExternalInput")
    out = nc.dram_tensor("out", [B, S_SH, D], f16, kind="ExternalOutput")
    v_ap, q_ap, o_ap = values.ap(), query.ap(), out.ap()

    with tile.TileContext(nc) as tc, ExitStack() as ctx:
        singles = ctx.enter_context(tc.tile_pool(name="singles", bufs=1))
        vpool = ctx.enter_context(tc.tile_pool(name="vpool", bufs=v_bufs))
        prodpool = ctx.enter_context(tc.tile_pool(name="prodpool", bufs=prod_bufs))
        wpool = ctx.enter_context(tc.tile_pool(name="wpool", bufs=6))
        pspool = ctx.enter_context(tc.tile_pool(name="pspool", bufs=4, space="PSUM"))
        qpool = ctx.enter_context(tc.tile_pool(name="qpool", bufs=2, space="PSUM"))

        # qtile[si*32 + b, :] = query[b, :]. The query is loaded from HBM
        # ONCE ([32, D], 0.18us of DMA, issued before the first values load
        # so it heads the DMA queue) and replicated 32->128 partitions
        # on-chip with one PE matmul against R = [I32 I32 I32 I32] (built by
        # GPSIMD affine_select: keep 1.0 where partition k == m%32). Reading
        # the 4x replica straight from HBM would cost 0.73us on the
        # exclusive DMA engines, which are the end-to-end bottleneck. The
        # broadcast finishes by ~4.5us, well before the first values unit
        # lands (~9.3us), so compute timing is unchanged.
        qtile = singles.tile([128, D], f16)
        q32 = singles.tile([B, D], f16)
        rtile = singles.tile([B, SG, B], f16)
        nc.gpsimd.memset(rtile, 1.0)
        nc.gpsimd.affine_select(
            rtile,
            rtile,
            pattern=[[0, SG], [-1, B]],
            compare_op=mybir.AluOpType.is_equal,
            fill=0.0,
            base=0,
            channel_multiplier=1,
        )
        def issue_qload():
            # placed at `qpos` among the leading unit loads: the HWDGE issues
            # DMAs 625ns apart, so a tiny transfer behind a small leading
            # load fills device idle instead of creating it
            nc.sync.dma_start(out=q32, in_=q_ap)
            half = D // 2
            for h in range(2):
                qb = qpool.tile([128, half], f32, tag="qb")
                nc.tensor.matmul(
                    qb, rtile, q32[:, h * half : (h + 1) * half],
                    start=True, stop=True,
                )
                nc.vector.tensor_copy(qtile[:, h * half : (h + 1) * half], qb)

        if qpos == 0:
            issue_qload()

        # Block-diagonal ones matrix: A[k, m] = 1 iff k//32 == m//32.
        # matmul(out, A, e) then computes out[p, j] = sum_{b in group(p)} e[b, j],
        # i.e. the group sum broadcast back to every partition of the group.
        atile = singles.tile([128, 128], f32)
        nc.vector.memset(atile, 0.0)
        for g in range(SG):
            nc.vector.memset(atile[g * B : (g + 1) * B, g * B : (g + 1) * B], 1.0)

        inv_sqrt_d = 1.0 / (float(np.sqrt(D)) * VSCALE)

        def do_wave(vtile, ounit, ujj, j_lo, j_hi):
            """Weights + scale + store for chunk range [j_lo, j_hi) of a
            loaded vtile; ounit is the matching output AP (or None until the
            final wave of the unit, which stores the whole unit)."""
            nw = j_hi - j_lo
            # dot products: wraw[p, j] = sum_d v[p, j, d] * q[b(p), d]
            # DVE elementwise product (fp16, 2x mode) + a fold-add of the last
            # `fold` elements onto the first `fold` (2x), then ScalarE
            # Copy-with-accumulate over the remaining D-fold elements.
            # (tensor_tensor_reduce faults on this HW; this split also
            # balances DVE vs ScalarE occupancy under the DMA roofline.)
            wraw = wpool.tile([128, nw], f32, tag="wraw")
            for j in range(j_lo, j_hi):
                prod = prodpool.tile([128, D], f16, tag="prod")
                nc.vector.tensor_mul(prod, vtile[:, j, :], qtile)
                if fold:
                    # the fold runs on the otherwise-idle GPSIMD engine (Add
                    # at 0.42 efficiency is still cheap), keeping both DVE
                    # and ScalarE below the per-unit DMA cadence
                    eng = nc.gpsimd if fold_engine == "pool" else nc.vector
                    eng.tensor_add(
                        prod[:, 0:fold], prod[:, 0:fold], prod[:, D - fold : D]
                    )
                red = prod[:, 0 : D - fold] if fold else prod
                nc.scalar.activation(
                    red,
                    red,
                    Act.Copy,
                    accum_out=wraw[:, j - j_lo : j - j_lo + 1],
                )

            # softmax over b (within each group of 32 partitions); the
            # 1/sqrt(D) score scale rides the Exp activation's scale input.
            etile = wpool.tile([128, nw], f32, tag="etile")
            nc.scalar.activation(etile, wraw, Act.Exp, scale=inv_sqrt_d)
            den = pspool.tile([128, nw], f32, tag="den")
            nc.tensor.matmul(den, atile, etile, start=True, stop=True)
            rec = wpool.tile([128, nw], f32, tag="rec")
            nc.vector.reciprocal(rec, den)
            wfin = wpool.tile([128, nw], f32, tag="wfin")
            nc.vector.tensor_mul(wfin, etile, rec)

            # scale values by the per-(b, s) weight, in place (vtile's last
            # reader is the dot-product mul, which already ran), and store.
            # fp16 tensor_scalar runs in the DVE 4x perf mode (the [128,1]
            # f32 scalar operand is exempt from the 2-byte requirement).
            for j in range(j_lo, j_hi):
                nc.vector.tensor_scalar_mul(
                    vtile[:, j, :], vtile[:, j, :],
                    wfin[:, j - j_lo : j - j_lo + 1]
                )
            if ounit is not None:
                # One 128-partition store for the whole unit on the ScalarE
                # HWDGE ring (qActDynamicHW) so its semaphore waits cannot
                # head-block the loads flowing through the SP ring.
                nc.scalar.dma_start(out=ounit, in_=vtile[:, 0:ujj, :])

        # unit sizes: uniform jj chunks, except tapered tail units (smaller
        # final units shorten the post-last-load compute+store tail)
        s_total = S_SH // SG  # total chunks per core
        tail = sum(taper) + sum(front)
        assert (s_total - tail) % jj == 0
        sizes = (list(front) + [jj] * ((s_total - tail) // jj)
                 + [t for t in taper if t])

        def run_waves(vtile, ounit, ujj):
            uw = wave or ujj
            for j_lo in range(0, ujj, uw):
                j_hi = min(j_lo + uw, ujj)
                do_wave(vtile, ounit if j_hi == ujj else None, ujj, j_lo, j_hi)

        # Software pipeline: the first `qpos` unit loads are issued before
        # the query load, so the tiny query transfer fills the early
        # HWDGE-issue-rate idle on the DMA engines instead of creating it;
        # compute waves are only emitted after the qtile writes exist in
        # program order (readers must follow writers for Tile's dependency
        # tracking).
        s0 = 0
        pend = []
        for ui, ujj in enumerate(sizes):
            vtile = vpool.tile([128, jj, D], f16, tag="vtile")
            # One DMA for the whole unit: AP dims (si, b, j, d) with (j d)
            # contiguous -> 128 descriptors of jj*D*2 = 16KB each.
            vunit = v_ap[:, s0 : s0 + SG * ujj, :].rearrange(
                "b (si j) d -> si b j d", si=SG, j=ujj
            )
            ounit = o_ap[:, s0 : s0 + SG * ujj, :].rearrange(
                "b (si j) d -> si b j d", si=SG, j=ujj
            )
            nc.sync.dma_start(out=vtile[:, 0:ujj, :], in_=vunit)
            pend.append((vtile, ounit, ujj))
            if ui + 1 == qpos:
                issue_qload()
            if ui + 1 >= qpos:
                run_waves(*pend.pop(0))
            s0 += SG * ujj
        for item in pend:
            run_waves(*item)

    nc.compile()
    return nc


def _get_nc():
    if "nc" not in _CACHE:
        _CACHE["nc"] = _build_nc()
    return _CACHE["nc"]


def kernel(query: np.ndarray, values: np.ndarray) -> np.ndarray:
    from concourse import bass_utils

    nc = _get_nc()
    # fp16 I/O: round-to-nearest on the host (values ~ N(0,1), far inside
    # fp16 range), upcast the result back to f32 after the gather.
    query = np.ascontiguousarray(np.asarray(query).astype(np.float16))
    values16 = (np.asarray(values) * np.float32(VSCALE)).astype(np.float16)
    in_maps = [
        {
            "values": np.ascontiguousarray(
                values16[:, c * S_SH : (c + 1) * S_SH, :]
            ),
            "query": query,
        }
        for c in range(N_CORES)
    ]
    last_exc = None
    for attempt in range(3):
        try:
            res = bass_utils.run_bass_kernel_spmd(
                nc, in_maps, core_ids=list(range(N_CORES))
            )
            out16 = np.concatenate([r["out"] for r in res.results], axis=1)
            return out16.astype(np.float32) * np.float32(1.0 / VSCALE)
        except ModuleNotFoundError:
            # BASS_TRACE=1 requests NTFF profiling, whose axon hook module is
            # not shipped in every container; fall back to an untraced run.
            os.environ["BASS_NEVER_TRACE"] = "1"
            last_exc = None
            continue
        except Exception as e:
            # A crashed previous run can leave a NeuronCore transiently
            # wedged (NRT_EXEC_UNIT_UNRECOVERABLE); NEURON_RT_RESET_CORES=1
            # recovers it on a fresh NRT session. Best effort: drop the jax
            # backend so the retry reconnects, and give the previous
            # session's teardown time to finish.
            last_exc = e
            import time as _time

            try:
                import jax.extend as _jex

                _jex.backend.clear_backends()
            except Exception:
                pass
            _time.sleep(20.0)
    raise last_exc
